# revision 1
# baseline (speedup 1.0000x reference)
"""Multi-head attention (B=4, S=2048, D=1024, H=16) on 8 trn2 NeuronCores.

Sharding: batch x query-half. Core c handles batch c//2, query rows
(c%2)*1024 : (c%2+1)*1024. Each core projects Q for its query chunk and K/V
for the full sequence of its batch (K/V projection duplicated across the two
cores sharing a batch), runs attention for all 16 heads, and applies the
output projection. No cross-core communication.

Device-side layout notes:
 - All activations are kept transposed ([feature, token]) so every matmul
   consumes operands directly: scores are computed as S^T[k,q] = K_h^T.T @ Q_h^T,
   softmax-exp runs on ScalarE, and the AV matmul contracts over k with
   lhsT = [V_h | ones-column] (stride 65), which makes row 64 of the PSUM
   output the softmax denominator. Normalization: DVE reciprocal + a 1x64
   PE matmul to replicate it across partitions + DVE multiply. No max
   subtraction (scores are O(+-5) here, fp32 exp is safe).
 - Head pairs (2h, 2h+1) sit in partitions 0:64 / 64:128 of the same tile;
   their QK matmuls use disjoint PE row-groups and run concurrently.
 - Both heads' scores land in one [128,1024] PSUM tile -> a single ScalarE
   exp instruction, halving ACT instruction overhead.
 - Matmul operands are bf16; accumulation is fp32 in PSUM.
 - Phase order: K proj, Q proj, then attention with the V projection fused
   into the first head-pair's loop and the qc0 output projection interleaved
   into qc1's attention — keeps TensorE fed while ScalarE runs exp.
 - V bias is folded into the output-projection bias host-side
   (softmax rows sum to 1 => attn @ (V + 1 b_v^T) = attn @ V + b_v^T).
"""

import numpy as np

B, S, D, H = 4, 2048, 1024, 16
DK = D // H          # 64
TQ = S // 2          # per-core query tokens
TK = S               # per-core key tokens
CW = 512             # x^T streaming chunk width (tokens)
N_CORES = 8
VP_W = H * (DK + 1)  # per head: 64 V columns + 1 ones column (stride 65)
SCALE = 1.0 / np.sqrt(DK)

_CACHE = {}


def _build_program(reps=1):
    import concourse.bass as bass
    import concourse.mybir as mybir
    from concourse import bacc
    from concourse.tile import TileContext

    f32 = mybir.dt.float32
    bf16 = mybir.dt.bfloat16
    AF = mybir.ActivationFunctionType

    nc = bacc.Bacc("TRN2", target_bir_lowering=False)

    xqT = nc.declare_dram_parameter("xqT", [D, TQ], bf16, isOutput=False)
    xkT = nc.declare_dram_parameter("xkT", [D, TK], bf16, isOutput=False)
    xvT = nc.declare_dram_parameter("xvT", [D, TK], bf16, isOutput=False)
    wqT = nc.declare_dram_parameter("wqT", [D, D], bf16, isOutput=False)
    wkT = nc.declare_dram_parameter("wkT", [D, D], bf16, isOutput=False)
    wvT = nc.declare_dram_parameter("wvT", [D, D], bf16, isOutput=False)
    woT = nc.declare_dram_parameter("woT", [D, D], bf16, isOutput=False)
    bq_in = nc.declare_dram_parameter("bq_in", [128, 8], f32, isOutput=False)
    bk_in = nc.declare_dram_parameter("bk_in", [128, 8], f32, isOutput=False)
    bo_in = nc.declare_dram_parameter("bo_in", [128, 8], f32, isOutput=False)
    yT = nc.declare_dram_parameter("yT", [D, TQ], f32, isOutput=True)

    # DRAM access helpers: feature dim split as (tile j, partition p)
    xq_r = xqT[:].rearrange("(a p) t -> p a t", p=128)
    xk_r = xkT[:].rearrange("(a p) t -> p a t", p=128)
    xv_r = xvT[:].rearrange("(a p) t -> p a t", p=128)
    wq_r = wqT[:].rearrange("(a p) d -> p a d", p=128)
    wk_r = wkT[:].rearrange("(a p) d -> p a d", p=128)
    wv_r = wvT[:].rearrange("(a p) d -> p a d", p=128)
    wo_r = woT[:].rearrange("(a p) d -> p a d", p=128)

    with TileContext(nc) as tc:
        for _rep in range(reps):
            _emit_body(nc, tc, bass, f32, bf16, AF,
                       xq_r, xk_r, xv_r, wq_r, wk_r, wv_r, wo_r,
                       bq_in, bk_in, bo_in, yT)
    nc.compile()
    return nc


def _emit_body(nc, tc, bass, f32, bf16, AF,
               xq_r, xk_r, xv_r, wq_r, wk_r, wv_r, wo_r,
               bq_in, bk_in, bo_in, yT):
    def mm(out, lhsT, rhs, start, stop):
        nc.tensor.matmul(out, lhsT=lhsT, rhs=rhs, start=start, stop=stop)

    if True:
        with (
            tc.tile_pool(name="const", bufs=1) as const_pool,
            tc.tile_pool(name="kt_res", bufs=1) as kt_pool,
            tc.tile_pool(name="qt_res", bufs=1) as qt_pool,
            tc.tile_pool(name="vp_res", bufs=1) as vp_pool,
            tc.tile_pool(name="ot_res", bufs=2) as ot_pool,
            tc.tile_pool(name="w_res", bufs=3) as w_pool,
            tc.tile_pool(name="x_str", bufs=3) as x_pool,
            tc.tile_pool(name="exp_p", bufs=4) as exp_pool,
            tc.tile_pool(name="rec_p", bufs=2) as rec_pool,
            tc.tile_pool(name="recb_p", bufs=3) as recb_pool,
            tc.tile_pool(name="oc_p", bufs=6) as oc_pool,
            tc.tile_pool(name="y_p", bufs=4) as y_pool,
            tc.tile_pool(name="ps_proj", bufs=2, space="PSUM") as ps_proj,
            tc.tile_pool(name="ps_s", bufs=2, space="PSUM") as ps_s,
            tc.tile_pool(name="ps_av", bufs=2, space="PSUM") as ps_av,
        ):
            bq_sb = const_pool.tile([128, 8], f32, tag="bq")
            bk_sb = const_pool.tile([128, 8], f32, tag="bk")
            bo_sb = const_pool.tile([128, 8], f32, tag="bo")
            # separate queue: keep these tiny loads off the head of the
            # sync queue that feeds the first matmuls
            nc.gpsimd.dma_start(out=bq_sb, in_=bq_in[:])
            nc.gpsimd.dma_start(out=bk_sb, in_=bk_in[:])
            nc.gpsimd.dma_start(out=bo_sb, in_=bo_in[:])

            KT_sb = kt_pool.tile([128, 8, TK], bf16, tag="KT")    # [p, j, t]
            QT_sb = qt_pool.tile([128, 8, TQ], bf16, tag="QT")    # [p, j, t]
            Vp_sb = vp_pool.tile([128, 16, VP_W], bf16, tag="Vp")  # [p, i, c]
            # view: [p, ktile, head, col(65)]
            Vp4 = Vp_sb.rearrange("p i (hh c) -> p i hh c", c=DK + 1)
            nc.vector.memset(Vp4[:, :, :, DK], 1.0)
            ones_sb = const_pool.tile([1, 64], bf16, tag="ones")
            nc.vector.memset(ones_sb, 1.0)

            # ---- K projection: K^T[dout, t], streamed x chunks ----
            # split loads so the first dj-column's matmuls start early
            wk_sb = w_pool.tile([128, 8, D], bf16, tag="wbig", name="wk_sb")
            nc.sync.dma_start(out=wk_sb[:, :, 0:128], in_=wk_r[:, :, 0:128])
            for tci in range(TK // CW):
                xc = x_pool.tile([128, 8, CW], bf16, tag="xchunk",
                                 name=f"xk_{tci}")
                if tci == 0:
                    for kq in range(4):
                        nc.sync.dma_start(
                            out=xc[:, 2 * kq:2 * kq + 2, :],
                            in_=xk_r[:, 2 * kq:2 * kq + 2, 0:CW])
                    # remaining K-weight columns, one dj-slice each, queued
                    # behind the first activation chunk
                    for djw in range(1, 8):
                        nc.sync.dma_start(
                            out=wk_sb[:, :, djw * 128:(djw + 1) * 128],
                            in_=wk_r[:, :, djw * 128:(djw + 1) * 128])
                else:
                    nc.sync.dma_start(out=xc,
                                      in_=xk_r[:, :, tci * CW:(tci + 1) * CW])
                for dj in range(8):
                    ps = ps_proj.tile([128, CW], f32, tag="pp",
                                      name=f"pk_{tci}_{dj}")
                    for kj in range(8):
                        mm(ps, wk_sb[:, kj, dj * 128:(dj + 1) * 128],
                           xc[:, kj, :], kj == 0, kj == 7)
                    nc.vector.tensor_scalar_add(
                        out=KT_sb[:, dj, tci * CW:(tci + 1) * CW],
                        in0=ps, scalar1=bk_sb[:, dj:dj + 1])

            # ---- Q projection ----
            wq_sb = w_pool.tile([128, 8, D], bf16, tag="wbig", name="wq_sb")
            nc.sync.dma_start(out=wq_sb, in_=wq_r)

            xq_tiles = {}

            def q_proj_dj(tci, dj):
                if dj == 0:
                    xq_tiles[tci] = x_pool.tile([128, 8, CW], bf16,
                                                tag="xchunk",
                                                name=f"xq_{tci}")
                    nc.sync.dma_start(
                        out=xq_tiles[tci],
                        in_=xq_r[:, :, tci * CW:(tci + 1) * CW])
                xc = xq_tiles[tci]
                ps = ps_proj.tile([128, CW], f32, tag="pp",
                                  name=f"pq_{tci}_{dj}")
                for kj in range(8):
                    mm(ps, wq_sb[:, kj, dj * 128:(dj + 1) * 128],
                       xc[:, kj, :], kj == 0, kj == 7)
                nc.vector.tensor_scalar_add(
                    out=QT_sb[:, dj, tci * CW:(tci + 1) * CW],
                    in0=ps, scalar1=bq_sb[:, dj:dj + 1])

            def q_proj_chunk(tci):
                for dj in range(8):
                    q_proj_dj(tci, dj)

            # V and Wo weights resident; V projection is fused into the first
            # head-pair's attention loop below so ScalarE exp overlaps it
            wv_sb = w_pool.tile([128, 8, D], bf16, tag="wbig", name="wv_sb")
            nc.sync.dma_start(out=wv_sb, in_=wv_r)
            wo_sb = w_pool.tile([128, 8, D], bf16, tag="wbig", name="wo_sb")
            nc.sync.dma_start(out=wo_sb, in_=wo_r)

            def v_proj_chunk(tci):
                xc = x_pool.tile([128, 8, CW], bf16, tag="xchunk",
                                 name=f"xv_{tci}")
                nc.sync.dma_start(out=xc,
                                  in_=xv_r[:, :, tci * CW:(tci + 1) * CW])
                for ts2 in range(CW // 128):
                    ti = tci * (CW // 128) + ts2
                    for dc in range(2):
                        ps = ps_proj.tile([128, CW], f32, tag="pp",
                                          name=f"pv_{ti}_{dc}")
                        for kj in range(8):
                            mm(ps[:, 0:512], xc[:, kj, ts2 * 128:(ts2 + 1) * 128],
                               wv_sb[:, kj, dc * 512:(dc + 1) * 512],
                               kj == 0, kj == 7)
                        nc.vector.tensor_copy(
                            out=Vp4[:, ti, dc * 8:(dc + 1) * 8, 0:DK],
                            in_=ps[:, 0:512].rearrange("p (hh c) -> p hh c", c=DK))

            OT_tiles = {}

            # deferred normalization: (qc, hp, sbuf copy of [65,512] accum)
            pending_norm = []

            def flush_norm():
                while pending_norm:
                    qc, hp, oc = pending_norm.pop(0)
                    for hh in range(2):
                        # row 64 of oc = softmax denominator
                        rec = rec_pool.tile([1, 512], bf16, tag="rec",
                                            name=f"rec_{qc}_{hp}_{hh}")
                        with nc.allow_low_precision(
                                reason="softmax denom reciprocal, bf16 "
                                       "matches pipeline precision"):
                            nc.vector.reciprocal(out=rec,
                                                 in_=oc[hh][64:65, :])
                        # replicate reciprocal across 64 partitions via PE
                        ps_rep = ps_proj.tile([128, CW], f32, tag="pp",
                                              name=f"pr_{qc}_{hp}_{hh}")
                        mm(ps_rep[0:64, 0:512], ones_sb, rec, True, True)
                        recb = recb_pool.tile([64, 512], f32, tag="recb",
                                              name=f"recb_{qc}_{hp}_{hh}")
                        nc.vector.tensor_copy(out=recb,
                                              in_=ps_rep[0:64, 0:512])
                        nc.vector.tensor_mul(
                            out=OT_tiles[qc][hh * 64:(hh + 1) * 64, hp, :],
                            in0=oc[hh][0:64, :], in1=recb)

            def attn_hp(qc, hp, fuse_v=False, fuse_k=None, fill=None):
                qsl = slice(qc * 512, (qc + 1) * 512)
                ps_o = [ps_av.tile([128, 512], f32, tag="po",
                                   name=f"po_{qc}_{hp}_{i}")
                        for i in range(2)]
                def qk_exp(kt):
                    # both heads' scores^T into one 2-bank PSUM tile
                    pss = ps_s.tile([128, 1024], f32, tag="pss",
                                    name=f"pss_{qc}_{hp}_{kt}")
                    for hh in range(2):
                        pb = hh * 64
                        mm(pss[:, hh * 512:(hh + 1) * 512],
                           KT_sb[pb:pb + 64, hp, kt * 128:(kt + 1) * 128],
                           QT_sb[pb:pb + 64, hp, qsl], True, True)
                    e = exp_pool.tile([128, 1024], bf16, tag="ex",
                                      name=f"ex_{qc}_{hp}_{kt}")
                    nc.scalar.activation(out=e, in_=pss, func=AF.Exp,
                                         scale=SCALE)
                    return e

                def av(kt, e):
                    for hh in range(2):
                        h = 2 * hp + hh
                        mm(ps_o[hh][0:65, :],
                           Vp_sb[:, kt, 65 * h:65 * h + 65],
                           e[:, hh * 512:(hh + 1) * 512],
                           kt == 0, kt == 15)

                # QK/exp run one kt ahead of AV so the pair-boundary
                # accumulator release is off the PE critical path
                e_prev = None
                for kt in range(16):
                    if fuse_v and kt % 4 == 0:
                        v_proj_chunk(kt // 4)
                    if fuse_k is not None and kt % 4 == 0:
                        k_proj_tci(fuse_k, kt // 4)
                    if fill and kt % 8 == 4:
                        fill.pop(0)()   # PE fill-in during ACT-bound stretch
                    if kt == 2:
                        # previous pair's normalization, now off the
                        # critical path (its PE replicate slots in here)
                        flush_norm()
                    e = qk_exp(kt)
                    if e_prev is not None:
                        av(kt - 1, e_prev)
                    e_prev = e
                av(15, e_prev)
                # copy accumulators (incl. denominator row) to SBUF right
                # away: frees both PSUM slots for the next pair's AVs
                oc = []
                for hh in range(2):
                    o_sb = oc_pool.tile([65, 512], f32, tag="oc",
                                        name=f"oc_{qc}_{hp}_{hh}")
                    nc.vector.tensor_copy(out=o_sb, in_=ps_o[hh][0:65, :])
                    oc.append(o_sb)
                pending_norm.append((qc, hp, oc))

            def wo_dj(qc, dj):
                qsl = slice(qc * 512, (qc + 1) * 512)
                ps_y = ps_proj.tile([128, CW], f32, tag="pp",
                                    name=f"py_{qc}_{dj}")
                for kj in range(8):
                    mm(ps_y[:, 0:512], wo_sb[:, kj, dj * 128:(dj + 1) * 128],
                       OT_tiles[qc][:, kj, :], kj == 0, kj == 7)
                yt = y_pool.tile([128, 512], f32, tag="yt",
                                 name=f"yt_{qc}_{dj}")
                nc.vector.tensor_scalar_add(
                    out=yt, in0=ps_y[:, 0:512], scalar1=bo_sb[:, dj:dj + 1])
                nc.sync.dma_start(
                    out=yT[dj * 128:(dj + 1) * 128, qsl], in_=yt)

            OT_tiles[0] = ot_pool.tile([128, 8, 512], bf16, tag="OT",
                                       name="OT_0")
            q_proj_chunk(0)        # QT for query-chunk 0
            # QT chunk 1 is produced as PE fill-in inside qc0's ACT-bound
            # head-pair loops (one dj-block per slot, hp1..hp4)
            fill_q = [lambda tci=1, dj=dj: q_proj_dj(tci, dj)
                      for dj in range(8)]
            for hp in range(8):
                fills = fill_q[2 * (hp - 1):2 * hp] if 1 <= hp <= 4 else None
                attn_hp(0, hp, fuse_v=(hp == 0), fill=fills)
            OT_tiles[1] = ot_pool.tile([128, 8, 512], bf16, tag="OT",
                                       name="OT_1")
            for hp in range(8):
                attn_hp(1, hp)
                if hp == 7:
                    # last pair's normalization before the final Wo block so
                    # its DVE chain hides under wo_dj(0,7)'s matmuls
                    flush_norm()
                wo_dj(0, hp)       # overlap qc0 output proj with qc1 attention
            for dj in range(8):
                wo_dj(1, dj)


def _prep_inputs(query, key, value, Wq, bq, Wk, bk, Wv, bv, Wo, bo):
    import ml_dtypes
    bf = ml_dtypes.bfloat16

    query = np.asarray(query, np.float32)
    key = np.asarray(key, np.float32)
    value = np.asarray(value, np.float32)
    wqT = np.ascontiguousarray(np.asarray(Wq, np.float32).T.astype(bf))
    wkT = np.ascontiguousarray(np.asarray(Wk, np.float32).T.astype(bf))
    wvT = np.ascontiguousarray(np.asarray(Wv, np.float32).T.astype(bf))
    woT = np.ascontiguousarray(np.asarray(Wo, np.float32).T.astype(bf))
    bo_eff = np.asarray(bo, np.float32) + \
        np.asarray(Wo, np.float32) @ np.asarray(bv, np.float32)
    bq_t = np.ascontiguousarray(np.asarray(bq, np.float32).reshape(8, 128).T)
    bk_t = np.ascontiguousarray(np.asarray(bk, np.float32).reshape(8, 128).T)
    bo_t = np.ascontiguousarray(bo_eff.reshape(8, 128).T)

    in_maps = []
    for c in range(N_CORES):
        b, qh = c // 2, c % 2
        in_maps.append({
            "xqT": np.ascontiguousarray(
                query[b, qh * TQ:(qh + 1) * TQ, :].T.astype(bf)),
            "xkT": np.ascontiguousarray(key[b].T.astype(bf)),
            "xvT": np.ascontiguousarray(value[b].T.astype(bf)),
            "wqT": wqT, "wkT": wkT, "wvT": wvT, "woT": woT,
            "bq_in": bq_t, "bk_in": bk_t, "bo_in": bo_t,
        })
    return in_maps


def kernel(query, key, value, Wq, bq, Wk, bk, Wv, bv, Wo, bo):
    from concourse.bass_utils import run_bass_kernel_spmd

    if "nc" not in _CACHE:
        _CACHE["nc"] = _build_program()
    nc = _CACHE["nc"]

    in_maps = _prep_inputs(query, key, value, Wq, bq, Wk, bk, Wv, bv, Wo, bo)
    res = run_bass_kernel_spmd(nc, in_maps, list(range(N_CORES)))
    out = np.empty((B, S, D), np.float32)
    for c in range(N_CORES):
        b, qh = c // 2, c % 2
        out[b, qh * TQ:(qh + 1) * TQ, :] = res.results[c]["yT"].T
    return out



# revision 4
# speedup vs baseline: 1.2587x; 1.2587x over previous
"""Multi-head attention (B=4, S=2048, D=1024, H=16) on 8 trn2 NeuronCores.

Sharding: batch x head-group (tensor parallel over heads). Core c handles
batch c//2 and heads (c%2)*8 .. (c%2)*8+7: it projects Q/K/V only for its
512 head dims (columns of Wq/Wk/Wv), runs attention for its 8 heads over
the full 2048-token sequence, and computes the PARTIAL output projection
y_g = O_g @ Wo[:, g-slice]^T (+ bias folded into group 0). The host adds
the two partials per batch during unshard - the row-sharded-Wo all-reduce
of standard tensor parallelism. No K/V projection duplication and no
cross-core traffic on device.

Device-side layout notes:
 - Activations stay transposed ([feature, token]); scores are computed as
   S^T[k, q] = K_h Q_h^T with head pairs stacked in partition halves
   (2 matmuls per 128-k tile, N=512 each), one ScalarE exp per [128,1024]
   PSUM tile.
 - AV is restructured for minimal PE row-streaming: the exp tile e[k, q]
   is the STATIONARY operand (lhsT, [128k x 128q] slices) and the moving
   operand is [V_h | ones] ([128k x 65]) so each matmul streams only 65
   rows -> out[q, 64+1] accumulates O[q, dk] AND the softmax denominator
   (col 64) over the 16 k-tiles. Per-partition reciprocal + tensor_scalar
   normalization (no cross-partition replicate needed), then a PE
   transpose (vs a DMA'd 128x128 identity) restores O^T[d, q] for the
   output projection.
 - Projections (K, Q, V, O) are emitted just-in-time inside the attention
   kt-loops to fill PE during ACT-bound stretches; a few warm-up matmuls
   on a zeroed scratch tile hold the PE p-state ramp while the first DMAs
   land.
 - V bias is folded into the (partial) output-projection bias host-side.
 - PSUM budget: scores 2x[128,1024] + AV accumulators 2x[128,512] +
   projection/transpose 2x[128,512] = 8 banks exactly.
"""

import numpy as np

B, S, D, H = 4, 2048, 1024, 16
DK = D // H          # 64
HL = H // 2          # 8 local heads per core
DG = HL * DK         # 512 local head dims
CW = 512             # token chunk width
QC = S // CW         # 4 query chunks
KTN = S // 128       # 16 k tiles
HPN = HL // 2        # 4 local head pairs
SCALE = 1.0 / np.sqrt(DK)
N_CORES = 8

_CACHE = {}


def _build_program(reps=1):
    import concourse.bass as bass
    import concourse.mybir as mybir
    from concourse import bacc
    from concourse.tile import TileContext

    f32 = mybir.dt.float32
    bf16 = mybir.dt.bfloat16
    AF = mybir.ActivationFunctionType

    nc = bacc.Bacc("TRN2", target_bir_lowering=False)

    xqT = nc.declare_dram_parameter("xqT", [D, S], bf16, isOutput=False)
    xkT = nc.declare_dram_parameter("xkT", [D, S], bf16, isOutput=False)
    xvT = nc.declare_dram_parameter("xvT", [D, S], bf16, isOutput=False)
    wqT = nc.declare_dram_parameter("wqT", [D, DG], bf16, isOutput=False)
    wkT = nc.declare_dram_parameter("wkT", [D, DG], bf16, isOutput=False)
    wvT = nc.declare_dram_parameter("wvT", [D, DG], bf16, isOutput=False)
    woT = nc.declare_dram_parameter("woT", [DG, D], bf16, isOutput=False)
    bq_in = nc.declare_dram_parameter("bq_in", [128, 4], f32, isOutput=False)
    bk_in = nc.declare_dram_parameter("bk_in", [128, 4], f32, isOutput=False)
    bo_in = nc.declare_dram_parameter("bo_in", [128, 8], f32, isOutput=False)
    id_in = nc.declare_dram_parameter("id_in", [128, 128], bf16,
                                      isOutput=False)
    yT = nc.declare_dram_parameter("yT", [D, S], f32, isOutput=True)

    xq_r = xqT[:].rearrange("(a p) t -> p a t", p=128)
    xk_r = xkT[:].rearrange("(a p) t -> p a t", p=128)
    xv_r = xvT[:].rearrange("(a p) t -> p a t", p=128)
    wq_r = wqT[:].rearrange("(a p) d -> p a d", p=128)
    wk_r = wkT[:].rearrange("(a p) d -> p a d", p=128)
    wv_r = wvT[:].rearrange("(a p) d -> p a d", p=128)
    wo_r = woT[:].rearrange("(a p) d -> p a d", p=128)

    with TileContext(nc) as tc:
        for _rep in range(reps):
            _emit_body(nc, tc, bass, f32, bf16, AF,
                       xq_r, xk_r, xv_r, wq_r, wk_r, wv_r, wo_r,
                       bq_in, bk_in, bo_in, id_in, yT)
    nc.compile()
    return nc


def _emit_body(nc, tc, bass, f32, bf16, AF,
               xq_r, xk_r, xv_r, wq_r, wk_r, wv_r, wo_r,
               bq_in, bk_in, bo_in, id_in, yT):
    def mm(out, lhsT, rhs, start, stop):
        nc.tensor.matmul(out, lhsT=lhsT, rhs=rhs, start=start, stop=stop)

    with (
        tc.tile_pool(name="const", bufs=1) as const_pool,
        tc.tile_pool(name="kt_res", bufs=1) as kt_pool,
        tc.tile_pool(name="qt_res", bufs=1) as qt_pool,
        tc.tile_pool(name="vp_res", bufs=1) as vp_pool,
        tc.tile_pool(name="w_res", bufs=1) as w_pool,
        tc.tile_pool(name="xk_p", bufs=4) as xk_pool,
        tc.tile_pool(name="xv_p", bufs=2) as xv_pool,
        tc.tile_pool(name="xq_p", bufs=2) as xq_pool,
        tc.tile_pool(name="exp_p", bufs=4) as exp_pool,
        tc.tile_pool(name="on_p", bufs=2) as on_pool,
        tc.tile_pool(name="rec_p", bufs=2) as rec_pool,
        tc.tile_pool(name="ot_res", bufs=2) as ot_pool,
        tc.tile_pool(name="y_p", bufs=4) as y_pool,
        tc.tile_pool(name="ps_proj", bufs=2, space="PSUM") as ps_proj,
        tc.tile_pool(name="ps_s", bufs=2, space="PSUM") as ps_s,
        tc.tile_pool(name="ps_o", bufs=2, space="PSUM") as ps_o,
    ):
        bq_sb = const_pool.tile([128, 4], f32, tag="bq")
        bk_sb = const_pool.tile([128, 4], f32, tag="bk")
        bo_sb = const_pool.tile([128, 8], f32, tag="bo")
        id_sb = const_pool.tile([128, 128], bf16, tag="ident")
        nc.gpsimd.dma_start(out=bq_sb, in_=bq_in[:])
        nc.gpsimd.dma_start(out=bk_sb, in_=bk_in[:])
        nc.gpsimd.dma_start(out=bo_sb, in_=bo_in[:])
        nc.gpsimd.dma_start(out=id_sb, in_=id_in[:])

        KT_sb = kt_pool.tile([128, 4, S], bf16, tag="KT")   # [p, dj, t]
        QT_sb = qt_pool.tile([128, 4, S], bf16, tag="QT")   # [p, dj, t]
        Vp_sb = vp_pool.tile([128, KTN, HL * (DK + 1)], bf16, tag="Vp")
        Vp4 = Vp_sb.rearrange("p i (hh c) -> p i hh c", c=DK + 1)
        nc.vector.memset(Vp4[:, :, :, DK], 1.0)

        # scratch for PE p-state warm-up (zeros; results unused)
        wsc = const_pool.tile([128, 640], bf16, tag="wsc")
        nc.vector.memset(wsc, 0.0)

        warm_ctr = [0]

        def warm(n):
            for _ in range(n):
                ps = ps_s.tile([128, 1024], f32, tag="pss",
                               name=f"warm_{warm_ctr[0]}")
                warm_ctr[0] += 1
                mm(ps[:, 0:512], wsc[:, 0:128], wsc[:, 128:640], True, True)

        wk_sb = w_pool.tile([128, 8, DG], bf16, tag="wk")
        wq_sb = w_pool.tile([128, 8, DG], bf16, tag="wq")
        wv_sb = w_pool.tile([128, 8, DG], bf16, tag="wv")
        wo_sb = w_pool.tile([128, 4, D], bf16, tag="wo")

        xk_t, xv_t, xq_t = {}, {}, {}

        def load_x(kind, tc_i):
            pool, cache, src = {
                "k": (xk_pool, xk_t, xk_r),
                "v": (xv_pool, xv_t, xv_r),
                "q": (xq_pool, xq_t, xq_r),
            }[kind]
            t = pool.tile([128, 8, CW], bf16, tag=f"x{kind}",
                          name=f"x{kind}_{tc_i}")
            nc.sync.dma_start(out=t, in_=src[:, :, tc_i * CW:(tc_i + 1) * CW])
            cache[tc_i] = t

        def k_proj(dj, tci):
            ps = ps_proj.tile([128, 512], f32, tag="pp",
                              name=f"pk_{dj}_{tci}")
            for kj in range(8):
                mm(ps, wk_sb[:, kj, dj * 128:(dj + 1) * 128],
                   xk_t[tci][:, kj, :], kj == 0, kj == 7)
            nc.vector.tensor_scalar_add(
                out=KT_sb[:, dj, tci * CW:(tci + 1) * CW],
                in0=ps, scalar1=bk_sb[:, dj:dj + 1])

        def q_proj(qc, dj):
            ps = ps_proj.tile([128, 512], f32, tag="pp",
                              name=f"pq_{qc}_{dj}")
            for kj in range(8):
                mm(ps, wq_sb[:, kj, dj * 128:(dj + 1) * 128],
                   xq_t[qc][:, kj, :], kj == 0, kj == 7)
            nc.vector.tensor_scalar_add(
                out=QT_sb[:, dj, qc * CW:(qc + 1) * CW],
                in0=ps, scalar1=bq_sb[:, dj:dj + 1])

        def v_proj(ti):
            tci, ts = ti // 4, ti % 4
            ps = ps_proj.tile([128, 512], f32, tag="pp", name=f"pv_{ti}")
            for kj in range(8):
                mm(ps, xv_t[tci][:, kj, ts * 128:(ts + 1) * 128],
                   wv_sb[:, kj, :], kj == 0, kj == 7)
            nc.vector.tensor_copy(
                out=Vp4[:, ti, :, 0:DK],
                in_=ps.rearrange("p (hh c) -> p hh c", c=DK))

        OT_tiles = {}

        def o_proj(qc, dj):
            ps_y = ps_proj.tile([128, 512], f32, tag="pp",
                                name=f"py_{qc}_{dj}")
            for kj in range(4):
                mm(ps_y, wo_sb[:, kj, dj * 128:(dj + 1) * 128],
                   OT_tiles[qc][:, kj, :], kj == 0, kj == 3)
            yt = y_pool.tile([128, 512], f32, tag="yt", name=f"yt_{qc}_{dj}")
            nc.vector.tensor_scalar_add(
                out=yt, in0=ps_y, scalar1=bo_sb[:, dj:dj + 1])
            nc.gpsimd.dma_start(
                out=yT[dj * 128:(dj + 1) * 128, qc * CW:(qc + 1) * CW],
                in_=yt)

        # deferred per-head-pair transposes: Onorm[q, d] -> OT[d, q]
        pending_tr = []

        def flush_tr():
            while pending_tr:
                qc, hp, onorm = pending_tr.pop(0)
                tp = ps_proj.tile([128, 512], f32, tag="pp",
                                  name=f"tp_{qc}_{hp}")
                tpb = tp[:, :].bitcast(bf16)
                for qb in range(4):
                    nc.tensor.matmul(tpb[:, qb * 128:(qb + 1) * 128],
                                     lhsT=onorm[:, qb, :], rhs=id_sb,
                                     start=True, stop=True,
                                     is_transpose=True)
                nc.vector.tensor_copy(out=OT_tiles[qc][:, hp, :],
                                      in_=tpb[:, 0:512])

        def attn_hp(qc, hp, fills=None):
            qsl = slice(qc * CW, (qc + 1) * CW)
            po = [ps_o.tile([128, 512], f32, tag="po",
                            name=f"po_{qc}_{hp}_{hh}") for hh in range(2)]

            def qk_exp(kt):
                pss = ps_s.tile([128, 1024], f32, tag="pss",
                                name=f"pss_{qc}_{hp}_{kt}")
                for hh in range(2):
                    pb = hh * 64
                    mm(pss[:, hh * 512:(hh + 1) * 512],
                       KT_sb[pb:pb + 64, hp, kt * 128:(kt + 1) * 128],
                       QT_sb[pb:pb + 64, hp, qsl], True, True)
                e = exp_pool.tile([128, 1024], bf16, tag="ex",
                                  name=f"ex_{qc}_{hp}_{kt}")
                nc.scalar.activation(out=e, in_=pss, func=AF.Exp,
                                     scale=SCALE)
                return e

            def av(kt, e):
                for hh in range(2):
                    h = 2 * hp + hh
                    for qb in range(4):
                        mm(po[hh][:, qb * 65:qb * 65 + 65],
                           e[:, hh * 512 + qb * 128:hh * 512 + (qb + 1) * 128],
                           Vp_sb[:, kt, h * 65:(h + 1) * 65],
                           kt == 0, kt == 15)

            e_prev = None
            for kt in range(KTN):
                if fills:
                    for th in fills.get(kt, []):
                        th()
                if kt == 2:
                    flush_tr()
                e = qk_exp(kt)
                if e_prev is not None:
                    av(kt - 1, e_prev)
                e_prev = e
            av(KTN - 1, e_prev)

            # drain: reciprocal of denominators (col 64 of each 65-block),
            # normalize into SBUF staging [q, d] (bf16)
            onorm = on_pool.tile([128, 4, 128], bf16, tag="on",
                                 name=f"on_{qc}_{hp}")
            rec = rec_pool.tile([128, 8], f32, tag="rec",
                                name=f"rec_{qc}_{hp}")
            for hh in range(2):
                nc.vector.reciprocal(out=rec[:, hh * 4:hh * 4 + 4],
                                     in_=po[hh][:, 64:260:65])
                for qb in range(4):
                    nc.vector.tensor_scalar_mul(
                        out=onorm[:, qb, hh * 64:(hh + 1) * 64],
                        in0=po[hh][:, qb * 65:qb * 65 + 64],
                        scalar1=rec[:, hh * 4 + qb:hh * 4 + qb + 1])
            pending_tr.append((qc, hp, onorm))

        # ---------------- schedule ----------------
        # prefix DMAs (sync queue order = arrival order on the DMA device)
        nc.sync.dma_start(out=wk_sb[:, :, 0:128], in_=wk_r[:, :, 0:128])
        load_x("k", 0)
        nc.sync.dma_start(out=wv_sb, in_=wv_r)
        load_x("v", 0)
        load_x("q", 0)
        nc.sync.dma_start(out=wq_sb[:, :, 0:128], in_=wq_r[:, :, 0:128])
        load_x("k", 1)

        warm(8)
        k_proj(0, 0)
        warm(3)
        v_proj(0)
        warm(2)
        q_proj(0, 0)
        warm(2)

        def dma_th(fn, *a):
            return lambda: fn(*a)

        def wrest(w_sb, w_r):
            return lambda: nc.sync.dma_start(out=w_sb[:, :, 128:DG],
                                             in_=w_r[:, :, 128:DG])

        for qc in range(QC):
            OT_tiles[qc] = ot_pool.tile([128, 4, 512], bf16, tag="OT",
                                        name=f"OT_{qc}")
            for hp in range(HPN):
                fills = {}
                if qc == 0:
                    if hp == 0:
                        # V just-in-time (one 128-token tile per kt), K dj0
                        # chunks, x prefetches, weight remainders
                        for kt in range(1, 16):
                            fills.setdefault(kt, []).append(
                                dma_th(v_proj, kt))
                        for kt in (4, 8, 12):
                            fills.setdefault(kt, []).insert(
                                0, dma_th(k_proj, 0, kt // 4))
                        fills.setdefault(0, []).insert(0, dma_th(load_x, "v", 1))
                        fills.setdefault(2, []).insert(0, dma_th(load_x, "k", 2))
                        fills.setdefault(4, []).insert(0, dma_th(load_x, "v", 2))
                        fills.setdefault(6, []).insert(0, dma_th(load_x, "k", 3))
                        fills.setdefault(8, []).insert(0, dma_th(load_x, "v", 3))
                        fills.setdefault(10, []).insert(0, wrest(wk_sb, wk_r))
                        fills.setdefault(12, []).insert(0, wrest(wq_sb, wq_r))
                        fills.setdefault(13, []).append(dma_th(q_proj, 0, 1))
                    else:
                        # K proj for the NEXT head pair's dj runs here JIT
                        for kt in range(0, 16, 4):
                            fills.setdefault(kt, []).append(
                                dma_th(k_proj, hp, kt // 4))
                        if hp < 3:
                            fills.setdefault(13, []).append(
                                dma_th(q_proj, 0, hp + 1))
                        if hp == 2:
                            fills.setdefault(2, []).insert(
                                0, dma_th(load_x, "q", 1))
                        if hp == 3:
                            fills.setdefault(2, []).insert(
                                0, lambda: nc.sync.dma_start(out=wo_sb,
                                                             in_=wo_r))
                            for i, kt in enumerate((5, 7, 9, 11)):
                                fills.setdefault(kt, []).append(
                                    dma_th(q_proj, 1, i))
                            fills.setdefault(13, []).insert(
                                0, dma_th(load_x, "q", 2))
                else:
                    fills.setdefault(4, []).append(
                        dma_th(o_proj, qc - 1, 2 * hp))
                    fills.setdefault(10, []).append(
                        dma_th(o_proj, qc - 1, 2 * hp + 1))
                    if qc < 3:
                        fills.setdefault(7, []).append(
                            dma_th(q_proj, qc + 1, hp))
                    if qc == 1 and hp == 3:
                        fills.setdefault(13, []).insert(
                            0, dma_th(load_x, "q", 3))
                attn_hp(qc, hp, fills)
        flush_tr()
        for dj in range(8):
            o_proj(3, dj)


def _prep_inputs(query, key, value, Wq, bq, Wk, bk, Wv, bv, Wo, bo):
    import ml_dtypes
    bf = ml_dtypes.bfloat16

    query = np.asarray(query, np.float32)
    key = np.asarray(key, np.float32)
    value = np.asarray(value, np.float32)
    Wq = np.asarray(Wq, np.float32)
    Wk = np.asarray(Wk, np.float32)
    Wv = np.asarray(Wv, np.float32)
    Wo = np.asarray(Wo, np.float32)
    bq = np.asarray(bq, np.float32)
    bk = np.asarray(bk, np.float32)
    bv = np.asarray(bv, np.float32)
    bo = np.asarray(bo, np.float32)

    ident = np.ascontiguousarray(np.eye(128, dtype=np.float32).astype(bf))

    xT = {}
    for b in range(B):
        xT[b] = (np.ascontiguousarray(query[b].T.astype(bf)),
                 np.ascontiguousarray(key[b].T.astype(bf)),
                 np.ascontiguousarray(value[b].T.astype(bf)))

    grp = {}
    for g in range(2):
        gs = slice(DG * g, DG * (g + 1))
        bo_eff = Wo[:, gs] @ bv[gs]
        if g == 0:
            bo_eff = bo_eff + bo
        grp[g] = {
            "wqT": np.ascontiguousarray(Wq.T[:, gs].astype(bf)),
            "wkT": np.ascontiguousarray(Wk.T[:, gs].astype(bf)),
            "wvT": np.ascontiguousarray(Wv.T[:, gs].astype(bf)),
            "woT": np.ascontiguousarray(Wo.T[gs, :].astype(bf)),
            "bq_in": np.ascontiguousarray(bq[gs].reshape(4, 128).T),
            "bk_in": np.ascontiguousarray(bk[gs].reshape(4, 128).T),
            "bo_in": np.ascontiguousarray(bo_eff.reshape(8, 128).T),
            "id_in": ident,
        }

    in_maps = []
    for c in range(N_CORES):
        b, g = c // 2, c % 2
        m = {"xqT": xT[b][0], "xkT": xT[b][1], "xvT": xT[b][2]}
        m.update(grp[g])
        in_maps.append(m)
    return in_maps


def kernel(query, key, value, Wq, bq, Wk, bk, Wv, bv, Wo, bo):
    from concourse.bass_utils import run_bass_kernel_spmd

    if "nc" not in _CACHE:
        _CACHE["nc"] = _build_program()
    nc = _CACHE["nc"]

    in_maps = _prep_inputs(query, key, value, Wq, bq, Wk, bk, Wv, bv, Wo, bo)
    res = run_bass_kernel_spmd(nc, in_maps, list(range(N_CORES)))
    out = np.empty((B, S, D), np.float32)
    for b in range(B):
        y = res.results[2 * b]["yT"] + res.results[2 * b + 1]["yT"]
        out[b] = y.T
    return out
